# revision 6
# baseline (speedup 1.0000x reference)
"""CRF loss (nn_CrfTagger) Trainium2 Bass kernel.

Full inputs in, full output out. Shards batch across 8 NeuronCores.

Per core (64 sequences, S=1024, T=64):
  log-partition: Z_b = 1^T M_1023 ... M_1 g_0 with M_s = diag(g_s) E^T,
  g_s = exp(logits_s - CSHIFT), E = exp(transitions).
  Positions are split into 16 chunks of 64 steps. Each chunk product T_c
  is numerically rank-1 (Birkhoff contraction ~3^-63), so
    Z = prod_{c=1..15} (v_c^T u_{c-1}) / prod_{c=1..14} (1^T u_c).
  u_c / v_c come from independent forward/backward vector recursions
  inside each chunk: 15 fwd + 15 bwd chains packed in a [128, 15*64]
  state slab advanced per slot by one stationary-weight matmul
  (W = blockdiag(E, E^T), fwd chains on partitions 0:64, bwd on 64:128)
  plus one PSUM*G elementwise multiply.
  Numerator (gold path): one-hot tiles via is_equal ((t, j) layout for
  the DVE 2x mode), contracted with PSUM-accumulated matmuls run as
  concurrent tile_position col-group pairs:
  emit = tr(sum_{b,j} L_j^T OH_j), trans = <sum OH_s^T OH_{s+1}, trans>.
"""

import os

import numpy as np
import ml_dtypes

B, S, T = 512, 1024, 64
NC_N = 8
BL = B // NC_N          # 64 sequences per core
NPAIR = 15              # chunk pairs (u-chunk p, y-chunk p+1)
NA, NB = 8, 7           # pairs in state slab A / B
NSLOT = 64              # steps per chunk
CSHIFT = 4.667
NWIN = 4                # G-slab DMA/exp windows
WSLOT = NSLOT // NWIN   # 16 slots per window

BF16 = ml_dtypes.bfloat16

_NC = None
_LAST = None


def _build():
    import concourse.bacc as bacc
    import concourse.bass as bass
    import concourse.tile as tile
    from concourse import mybir

    f32 = mybir.dt.float32
    bf = mybir.dt.bfloat16
    AF = mybir.ActivationFunctionType
    AL = mybir.AluOpType
    AX = mybir.AxisListType

    nc = bacc.Bacc("TRN2", target_bir_lowering=False, debug=False, num_devices=NC_N)

    # graw[t, w, p, k, b]      = logits[b, 64p + 16w + k, t]        (fwd rows)
    # graw[64+t, w, p, k, b]   = logits[b, 64p + 127 - 16w - k, t]  (bwd rows)
    # single contiguous stream: 30KB/partition per window DMA
    graw = nc.dram_tensor("graw", [128, NWIN, NPAIR, WSLOT, BL], bf,
                          kind="ExternalInput")
    # lgb2[p, gi, g, t*8+j] = logits[gi*8+g, 8p+j, t]  ((t, j) layout, 8KB/part)
    lgb2 = nc.dram_tensor("lgb2", [128, 8, 8, 512], bf, kind="ExternalInput")
    tg8 = nc.dram_tensor("tg8", [128, BL * 8], bf, kind="ExternalInput")
    tbp = nc.dram_tensor("tbp", [128, BL], bf, kind="ExternalInput")
    tbn = nc.dram_tensor("tbn", [128, BL], bf, kind="ExternalInput")
    trs = nc.dram_tensor("trs", [T, T], f32, kind="ExternalInput")
    trsT = nc.dram_tensor("trsT", [T, T], f32, kind="ExternalInput")
    trp = nc.dram_tensor("trp", [128, T], f32, kind="ExternalInput")
    iot = nc.dram_tensor("iot", [128, 512], bf, kind="ExternalInput")
    trm = nc.dram_tensor("trm", [128, T], bf, kind="ExternalInput")
    out_loss = nc.dram_tensor("loss", [1, 1], f32, kind="ExternalOutput")
    out_dbg = nc.dram_tensor("dbg", [4, BL], f32, kind="ExternalOutput")

    def bcast_ap(ap, dims, extra_off=0):
        return bass.AP(tensor=ap.tensor, offset=ap.offset + extra_off,
                       ap=[ap.ap[0]] + dims)

    with tile.TileContext(nc) as tc:
        with (
            tc.tile_pool(name="cst", bufs=1) as cst,
            tc.tile_pool(name="numer", bufs=2) as nmr,
            tc.tile_pool(name="accps", bufs=1, space="PSUM") as accp,
        ):
            # ---------------- constants -----------------
            w_sb = cst.tile([128, 128], bf, tag="w")
            nc.vector.memset(w_sb[:], 0.0)
            trs_sb = cst.tile([T, T], f32, tag="trs")
            trsT_sb = cst.tile([128, T], f32, tag="trsT")
            nc.sync.dma_start(out=trs_sb[:], in_=trs[:])
            nc.sync.dma_start(out=trsT_sb[64:128, :], in_=trsT[:])
            # W = blockdiag(E, E^T) in bf16
            nc.scalar.activation(out=w_sb[0:64, 0:64], in_=trs_sb[:], func=AF.Exp)
            nc.scalar.activation(out=w_sb[64:128, 64:128], in_=trsT_sb[64:128, :],
                                 func=AF.Exp)

            iot_sb = cst.tile([128, 512], bf, tag="iot")
            nc.sync.dma_start(out=iot_sb[:], in_=iot[:])
            trm_sb = cst.tile([128, T], bf, tag="trm")
            nc.sync.dma_start(out=trm_sb[:], in_=trm[:])
            trp_sb = cst.tile([128, T], f32, tag="trp")
            nc.sync.dma_start(out=trp_sb[:], in_=trp[:])
            tg8_sb = cst.tile([128, BL * 8], bf, tag="tg8")
            nc.sync.dma_start(out=tg8_sb[:], in_=tg8[:])
            tbp_sb = cst.tile([128, BL], bf, tag="tbp")
            nc.sync.dma_start(out=tbp_sb[:], in_=tbp[:])
            tbn_sb = cst.tile([128, BL], bf, tag="tbn")
            nc.sync.dma_start(out=tbn_sb[:], in_=tbn[:])

            shift_sb = cst.tile([128, 1], f32, tag="shift")
            nc.vector.memset(shift_sb[:], -CSHIFT)
            ones_b = cst.tile([T, 1], bf, tag="onesb")
            nc.vector.memset(ones_b[:], 1.0)
            ones_f = cst.tile([T, 1], f32, tag="onesf")
            nc.vector.memset(ones_f[:], 1.0)
            ones128 = cst.tile([128, 1], f32, tag="ones128")
            nc.vector.memset(ones128[:], 1.0)

            # colsumE = E^T 1  (per-partition scalar for u-chain inits)
            cs_ps = accp.tile([T, 1], f32, tag="csps")
            nc.tensor.matmul(cs_ps[:], lhsT=w_sb[0:64, 0:64], rhs=ones_b[:],
                             start=True, stop=True)
            cse_sb = cst.tile([T, 1], f32, tag="cse")
            nc.vector.tensor_copy(out=cse_sb[:], in_=cs_ps[:])

            # boundary one-hot slabs [128, T*BL] ((t, b) layout)
            ohp_sb = cst.tile([128, T * BL], bf, tag="ohp")
            ohn_sb = cst.tile([128, T * BL], bf, tag="ohn")
            iot64_b = bcast_ap(iot_sb[:], [[8, T], [0, BL]])   # value t, any j col
            nc.vector.tensor_tensor(out=ohp_sb[:],
                                    in0=bcast_ap(tbp_sb[:], [[0, T], [1, BL]]),
                                    in1=iot64_b, op=AL.is_equal)
            nc.vector.tensor_tensor(out=ohn_sb[:],
                                    in0=bcast_ap(tbn_sb[:], [[0, T], [1, BL]]),
                                    in1=iot64_b, op=AL.is_equal)

            # ---------------- G slabs (one tile per window) ----------------
            # [128, pair, wslot, b]; rows 0:64 fwd-u copies, 64:128 bwd-y.
            gslw = [cst.tile([128, NPAIR, WSLOT, BL], bf, tag=f"gsl{w}",
                             name=f"gsl{w}")
                    for w in range(NWIN)]
            # state slabs: A = pairs 0..7, B = pairs 8..14
            sta = cst.tile([128, NA, BL], bf, tag="sta")
            stb = cst.tile([128, NB, BL], bf, tag="stb")

            # numerator accumulators ([128, T]: col-group 0 on partitions
            # 0:64, col-group 1 on 64:128 via tile_position)
            emit_ps = accp.tile([128, T], f32, tag="emitps")
            cmat_ps = accp.tile([128, T], f32, tag="cmatps")

            def g_window(w):
                g = gslw[w]
                nc.sync.dma_start(out=g[:], in_=graw[:, w, :, :, :])
                nc.scalar.activation(out=g[:], in_=g[:], func=AF.Exp,
                                     bias=shift_sb[:], scale=1.0)

            # numerator accumulation-group bookkeeping
            nseq = {"e0": 256, "e1": 256, "c0": 256, "c1": 256}
            nidx = {k: 0 for k in nseq}

            def acc_mm(key, out_ap, lhsT, rhs, pos):
                i = nidx[key]
                nidx[key] += 1
                nc.tensor.matmul(out_ap, lhsT=lhsT, rhs=rhs,
                                 start=(i == 0), stop=(i == nseq[key] - 1),
                                 skip_group_check=True, tile_position=pos)

            # lf slabs: 8 sequences per DMA (scalar-engine HWDGE queue)
            lfg = [None] * 8

            def load_lf_group(gi):
                t = nmr.tile([128, 8, 512], bf, tag="lfg", name=f"lfg{gi}")
                nc.scalar.dma_start(out=t[:], in_=lgb2[:, gi, :, :])
                lfg[gi] = t

            def jsl(ap2d, j, off=0):
                # (t, j)-layout column slice: [128, 64] strided view, col t*8+j
                return bcast_ap(ap2d, [[8, T]], extra_off=j + off)

            def numer_b(b):
                lf = lfg[b // 8][:, b % 8, :]
                oh = nmr.tile([128, 512], bf, tag="oh", bufs=4)
                tgb = bcast_ap(tg8_sb[:], [[0, T], [1, 8]], extra_off=b * 8)
                nc.vector.tensor_tensor(out=oh[:], in0=tgb, in1=iot_sb[:],
                                        op=AL.is_equal)
                for j in range(8):
                    g = j % 2
                    acc_mm("e" + str(g), emit_ps[g * 64:g * 64 + 64, :],
                           jsl(lf, j), jsl(oh[:], j), (0, g * 64))
                for j in range(7):
                    g = j % 2
                    acc_mm("c" + str(g), cmat_ps[g * 64:g * 64 + 64, :],
                           jsl(oh[:], j), jsl(oh[:], j + 1), (0, g * 64))
                bsl = bcast_ap(ohp_sb[:], [[BL, T]], extra_off=b)
                bsr = bcast_ap(ohn_sb[:], [[BL, T]], extra_off=b)
                acc_mm("c1", cmat_ps[64:128, :], bsl, bsr, (0, 64))

            # ---------------- main loop ----------------
            g_window(0)
            load_lf_group(0)
            g0 = gslw[0]
            # init states from slot 0:
            #   u-chains: w = colsumE * g'(64p)   (pair 0: w = g'_0)
            #   y-chains: y = g'(64(p+1)+63)
            nc.vector.tensor_scalar(out=sta[0:64, :, :], in0=g0[0:64, 0:NA, 0, :],
                                    scalar1=cse_sb[:], scalar2=None, op0=AL.mult)
            nc.vector.tensor_scalar(out=stb[0:64, :, :], in0=g0[0:64, NA:NPAIR, 0, :],
                                    scalar1=cse_sb[:], scalar2=None, op0=AL.mult)
            nc.vector.tensor_copy(out=sta[0:64, 0, :], in_=g0[0:64, 0, 0, :])
            nc.vector.tensor_copy(out=sta[64:128, :, :], in_=g0[64:128, 0:NA, 0, :])
            nc.vector.tensor_copy(out=stb[64:128, :, :], in_=g0[64:128, NA:NPAIR, 0, :])

            nb_done = 0

            def drain_numer(upto):
                nonlocal nb_done
                while nb_done < upto:
                    gi = nb_done // 8
                    if nb_done % 8 == 0 and gi + 1 < 8 and lfg[gi + 1] is None:
                        load_lf_group(gi + 1)
                    numer_b(nb_done)
                    nb_done += 1

            drain_numer(6)

            with tc.tile_pool(name="mainps", bufs=1, space="PSUM") as mp:
                vu_sb = cst.tile([T, NPAIR * BL], f32, tag="vu")
                for i in range(1, NSLOT):
                    w, k = divmod(i, WSLOT)
                    if i % WSLOT == 1 and w + 1 < NWIN:
                        g_window(w + 1)
                    g = gslw[w]
                    psa = mp.tile([128, NA * BL], f32, tag="psa")
                    psb = mp.tile([128, NB * BL], f32, tag="psb")
                    nc.tensor.matmul(psa[:], lhsT=w_sb[:], rhs=sta[:],
                                     start=True, stop=True)
                    nc.tensor.matmul(psb[:], lhsT=w_sb[:], rhs=stb[:],
                                     start=True, stop=True)
                    nc.vector.tensor_tensor(out=sta[:], in0=psa[:],
                                            in1=g[:, 0:NA, k, :], op=AL.mult)
                    nc.vector.tensor_tensor(out=stb[:], in0=psb[:],
                                            in1=g[:, NA:NPAIR, k, :], op=AL.mult)
                    # spread numerator work across the slot loop
                    drain_numer(min(BL, 6 + (i * (BL - 6)) // NSLOT))

                drain_numer(BL)

                # v_c = E y_c: lhsT rows 64:128 (E^T block), out partitions 0:64
                vps_a = mp.tile([T, NA * BL], f32, tag="psa")
                vps_b = mp.tile([T, NB * BL], f32, tag="psb")
                nc.tensor.matmul(vps_a[:], lhsT=w_sb[64:128, 64:128],
                                 rhs=sta[64:128, :, :], start=True, stop=True)
                nc.tensor.matmul(vps_b[:], lhsT=w_sb[64:128, 64:128],
                                 rhs=stb[64:128, :, :], start=True, stop=True)
                # vu = v * u elementwise (fp32)
                nc.vector.tensor_tensor(out=vu_sb[:, 0:NA * BL], in0=vps_a[:],
                                        in1=sta[0:64, :, :], op=AL.mult)
                nc.vector.tensor_tensor(out=vu_sb[:, NA * BL:NPAIR * BL],
                                        in0=vps_b[:],
                                        in1=stb[0:64, :, :], op=AL.mult)

            # ---------------- reductions / final ----------------
            with tc.tile_pool(name="postps", bufs=1, space="PSUM") as pp:
                ip_ps = pp.tile([1, NPAIR * BL], f32, tag="ipps")
                nc.tensor.matmul(ip_ps[:, 0:512], lhsT=ones_f[:],
                                 rhs=vu_sb[:, 0:512], start=True, stop=True)
                nc.tensor.matmul(ip_ps[:, 512:NPAIR * BL], lhsT=ones_f[:],
                                 rhs=vu_sb[:, 512:NPAIR * BL], start=True, stop=True)
                # s_c = 1^T u_c for c = 1..14: slab A pairs 1..7, slab B pairs 0..6
                sc_ps = pp.tile([1, 1024], f32, tag="scps")
                nc.tensor.matmul(sc_ps[:, 0:448], lhsT=ones_b[:],
                                 rhs=sta[0:64, 1:NA, :], start=True, stop=True)
                nc.tensor.matmul(sc_ps[:, 512:960], lhsT=ones_b[:],
                                 rhs=stb[0:64, 0:NB, :], start=True, stop=True)

                ip_lg = cst.tile([1, NPAIR * BL], f32, tag="iplg")
                sc_lg = cst.tile([1, 1024], f32, tag="sclg")
                nc.scalar.activation(out=ip_lg[:], in_=ip_ps[:], func=AF.Ln)
                nc.scalar.activation(out=sc_lg[:, 0:448], in_=sc_ps[:, 0:448],
                                     func=AF.Ln)
                nc.scalar.activation(out=sc_lg[:, 512:960], in_=sc_ps[:, 512:960],
                                     func=AF.Ln)

                ipr = cst.tile([1, BL], f32, tag="ipr")
                nc.vector.reduce_sum(
                    ipr[:], ip_lg[:].rearrange("p (c b) -> p b c", c=NPAIR),
                    axis=AX.X)
                sc1 = cst.tile([1, BL], f32, tag="sc1")
                sc2 = cst.tile([1, BL], f32, tag="sc2")
                nc.vector.reduce_sum(
                    sc1[:], sc_lg[:, 0:448].rearrange("p (c b) -> p b c", c=7),
                    axis=AX.X)
                nc.vector.reduce_sum(
                    sc2[:], sc_lg[:, 512:960].rearrange("p (c b) -> p b c", c=7),
                    axis=AX.X)
                scr = cst.tile([1, BL], f32, tag="scr")
                nc.vector.tensor_tensor(out=scr[:], in0=sc1[:], in1=sc2[:],
                                        op=AL.add)
                dif = cst.tile([1, BL], f32, tag="dif")
                nc.vector.tensor_tensor(out=dif[:], in0=ipr[:], in1=scr[:],
                                        op=AL.subtract)
                dtot = cst.tile([1, 1], f32, tag="dtot")
                nc.vector.reduce_sum(dtot[:], dif[:], axis=AX.X)

                # numerator extraction (emit/cmat: [128, T] two col-groups)
                etr = cst.tile([128, T], f32, tag="etr")
                nc.vector.tensor_tensor(out=etr[:], in0=emit_ps[:], in1=trm_sb[:],
                                        op=AL.mult)
                ctr = cst.tile([128, T], f32, tag="ctr")
                nc.vector.tensor_tensor(out=ctr[:], in0=cmat_ps[:], in1=trp_sb[:],
                                        op=AL.mult)
                ev = cst.tile([128, 1], f32, tag="ev")
                cv = cst.tile([128, 1], f32, tag="cv")
                nc.vector.reduce_sum(ev[:], etr[:], axis=AX.X)
                nc.vector.reduce_sum(cv[:], ctr[:], axis=AX.X)
                nv = cst.tile([128, 1], f32, tag="nv")
                nc.vector.tensor_tensor(out=nv[:], in0=ev[:], in1=cv[:], op=AL.add)
                num_ps = pp.tile([1, 1], f32, tag="numps")
                nc.tensor.matmul(num_ps[:], lhsT=nv[:], rhs=ones128[:],
                                 start=True, stop=True)
                nsb = cst.tile([1, 1], f32, tag="nsb")
                nc.vector.tensor_copy(out=nsb[:], in_=num_ps[:])

                loss_sb = cst.tile([1, 1], f32, tag="losssb")
                nc.vector.tensor_tensor(out=loss_sb[:], in0=nsb[:], in1=dtot[:],
                                        op=AL.subtract)
                nc.vector.tensor_scalar_add(loss_sb[:], loss_sb[:],
                                            float(-BL * S * CSHIFT))
                nc.sync.dma_start(out=out_loss[:], in_=loss_sb[:])
                nc.sync.dma_start(out=out_dbg[0:1, :], in_=ipr[:])
                nc.sync.dma_start(out=out_dbg[1:2, :], in_=scr[:])
                nc.sync.dma_start(out=out_dbg[2:3, :], in_=dif[:])
                nc.sync.dma_start(out=out_dbg[3:4, :], in_=dif[:])

    nc.finalize()
    return nc


def _marshal(logits, transitions, tags):
    """Per-core input dicts (host-side sharding/layout only)."""
    lg = np.asarray(logits)
    tg = np.asarray(tags).astype(np.int64)
    tr = np.asarray(transitions).astype(np.float32)

    trp = np.ascontiguousarray(np.concatenate([tr, tr], axis=0), np.float32)
    # iot[p, t*8+j] = t
    iot = np.repeat(np.arange(T, dtype=np.float32), 8)[None, :].repeat(128, 0)
    iot = np.ascontiguousarray(iot.astype(BF16))
    eye = np.eye(T, dtype=np.float32)
    trm = np.ascontiguousarray(np.concatenate([eye, eye], axis=0).astype(BF16))
    trsT = np.ascontiguousarray(tr.T)
    # graw gather indices [w, p, k]: fwd = 64p+16w+k, bwd = 64p+127-16w-k
    wg, pg, kg = np.meshgrid(np.arange(NWIN), np.arange(NPAIR),
                             np.arange(WSLOT), indexing="ij")
    fw_idx = (64 * pg + WSLOT * wg + kg).reshape(-1)
    bw_idx = (64 * pg + 127 - WSLOT * wg - kg).reshape(-1)

    in_maps = []
    for c in range(NC_N):
        bsl = slice(c * BL, (c + 1) * BL)
        lgc = lg[bsl].astype(BF16)                          # [BL, S, T]
        lgt = np.ascontiguousarray(lgc.transpose(2, 1, 0))  # [T, S, BL]
        graw = np.concatenate([lgt[:, fw_idx, :], lgt[:, bw_idx, :]], axis=0)
        graw = np.ascontiguousarray(
            graw.reshape(128, NWIN, NPAIR, WSLOT, BL))
        # lgb2: [p, gi, g, (t, j)]
        lgb = lgc.reshape(BL, 128, 8, T).transpose(1, 0, 3, 2)   # [p, b, t, j]
        lgb2 = np.ascontiguousarray(lgb.reshape(128, 8, 8, 512))
        tgc = tg[bsl]                                       # [BL, S]
        # tg8[p, b*8+j] = tag[b, 8p+j]
        t8 = tgc.reshape(BL, 128, 8).transpose(1, 0, 2).reshape(128, BL * 8)
        tg8 = np.ascontiguousarray(t8.astype(np.float32).astype(BF16))
        # boundary pair tags (p<=126); -1 padding kills the one-hot
        tbp = np.full((128, BL), -1.0, np.float32)
        tbn = np.full((128, BL), -1.0, np.float32)
        tbp[:127, :] = tgc[:, 7::8].T[:127]    # tag[b, 8p+7]
        tbn[:127, :] = tgc[:, 8::8].T          # tag[b, 8p+8], 127 cols
        in_maps.append({
            "graw": graw,
            "lgb2": lgb2,
            "tg8": tg8,
            "tbp": np.ascontiguousarray(tbp.astype(BF16)),
            "tbn": np.ascontiguousarray(tbn.astype(BF16)),
            "trs": tr,
            "trsT": trsT,
            "trp": trp,
            "iot": iot,
            "trm": trm,
        })
    return in_maps


def kernel(logits, transitions, tags, mask):
    global _NC, _LAST
    from concourse.bass_utils import run_bass_kernel_spmd

    assert np.asarray(mask).all(), "kernel assumes mask of all ones"
    if _NC is None:
        _NC = _build()
    in_maps = _marshal(logits, transitions, tags)
    res = run_bass_kernel_spmd(
        _NC, in_maps, core_ids=list(range(NC_N)),
        trace=os.environ.get("CRF_TRACE") == "1")
    _LAST = res
    total = np.float64(0.0)
    for c in range(NC_N):
        total += np.float64(res.results[c]["loss"][0, 0])
    return np.float32(total)

